# revision 6
# baseline (speedup 1.0000x reference)
"""Trainium2 Bass kernel for nn_MixtureAlignmentLogLikelihood.

Math: with trg_p = softmax(trg_sent, axis=2), every row of trg_p sums to 1
and P_st is the uniform matrix 1/Kt, so dot[b, t] = 1/Kt exactly and

  log_likelihood = -log(Kt) * sum(scales)

sum(scales) depends only on trg_boundary: per batch row (T positions,
boundary bits z in {0,1}):

  count = popcount(z); first = z[0]; lastp1 = (last set index)+1 (0 if none)
  sum_scales = count - first - max(lastp1, 1) + T + 1

Device kernel (per core): the 32 batch rows are laid out as 128 SBUF
partitions x 512 free (4 chunks of 512 per row, chunk j of row r on
partition 4r+j).  Per chunk the device computes
  m_j   = max_i  z[i] * (i+1)          (local lastp1 within the chunk)
  cnt_j = sum_i  z[i]                  (chunk popcount)
via ONE fused DVE tensor_tensor_reduce (mult+max) and ONE scalar-engine
activation accumulate running in parallel.  The [128,2] result is DMA'd
out; the O(B) chunk combine (lastp1 = max_j [m_j>0](m_j + 512j), count =
sum_j cnt_j) plus the scalar all-reduce over rows/cores happens on the
host during the gather, as does reading first = z[0] directly from the
input.  All quantities are small integers -> exact in int16/f32.

Schedule: no nc.Block() -- instructions are emitted at top level so the
input DMA is the very first post-preamble instruction on each queue.  The
input is split across both HWDGE queues (SP + Activation) so the two
halves transfer concurrently; the gpsimd iota (free dim 512) and the
scalar engine's ACT table load (hoisted via a dummy activation on a
framework const AP) overlap the input DMA.  A single semaphore S
sequences everything: each input DMA +16, iota +16 (consumers wait 48);
DVE +1 and scalar-accum +1 (output DMA waits 50).  The final 1KB output
DMA is not engine-waited: NEFF completion semantics (engine halt + DGE
queue quiesce in the runtime) cover it, which was verified empirically
over repeated randomized runs of the previous kernel revision.
"""

import math

import numpy as np

B, T, K = 256, 2048, 64
N_CORES = 8
BS = B // N_CORES  # 32 batch rows per core
NCHUNK = 4
CH = T // NCHUNK  # 512
NEG_LOG_K = -math.log(float(K))

_CACHE: dict = {}


def _build_nc(final_wait: bool = False):
    import concourse.bass as bass
    import concourse.mybir as mybir

    f32 = mybir.dt.float32
    i16 = mybir.dt.int16
    i8 = mybir.dt.int8

    P = BS * NCHUNK  # 128 partitions

    nc = bass.Bass(enable_partition_id=False, monotonic_sem_count=0)
    tb = nc.dram_tensor("tb", [P, CH], i8, kind="ExternalInput")
    out = nc.dram_tensor("out", [P, 2], f32, kind="ExternalOutput")

    H = P // 2  # 64 partitions per DMA queue

    with (
        nc.sbuf_tensor("tbs", [P, CH], i8) as tbs,
        nc.sbuf_tensor("iot", [P, CH], i16) as iot,
        nc.sbuf_tensor("prod", [P, CH], f32) as prod,
        nc.sbuf_tensor("adum", [P, CH], i8) as adum,
        nc.sbuf_tensor("dum1", [P, 1], f32) as dum1,
        nc.sbuf_tensor("outb", [P, 2], f32) as outb,
        nc.semaphore("s") as s,
    ):
        c0 = nc.const_aps.aps[(f32, 0.0)]

        # Input DMA halves on both HWDGE queues -- first post-preamble
        # instruction on each engine so the transfers start ASAP.
        nc.sync.dma_start(tbs[0:H, :], tb[0:H, :]).then_inc(s, 16)
        nc.scalar.dma_start(tbs[H:P, :], tb[H:P, :]).then_inc(s, 16)

        # Index vector 1..CH on every partition; overlaps the input DMA.
        nc.gpsimd.iota(
            iot[:], pattern=[[1, CH]], base=1, channel_multiplier=0
        ).then_inc(s, 16)

        # Dummy activation on a framework const AP: hoists the 1.3us
        # ACT_TABLE_LOAD into the DMA window instead of after it.
        nc.scalar.activation(dum1[:], c0, mybir.ActivationFunctionType.Copy)

        # cnt_j = add-accumulate of Copy(tb); f32 accum of 0/1 ints is exact
        nc.scalar.wait_ge(s, 48)
        nc.scalar.activation(
            adum[:],
            tbs[:],
            mybir.ActivationFunctionType.Copy,
            accum_out=outb[:, 1:2],
        ).then_inc(s, 1)

        # m_j = max_i tb[i]*(i+1)  (tensor_tensor_reduce does this in one
        # instruction but this walrus version rejects its ISA encoding)
        nc.vector.wait_ge(s, 48)
        nc.vector.tensor_tensor(
            prod[:], tbs[:], iot[:], op=mybir.AluOpType.mult
        ).then_inc(s, 1)
        # reduce depends on the mult only; same-engine program order covers it
        nc.vector.tensor_reduce(
            outb[:, 0:1], prod[:], axis=mybir.AxisListType.X, op=mybir.AluOpType.max
        ).then_inc(s, 1)

        nc.sync.wait_ge(s, 51)
        nc.sync.dma_start(out[:, :], outb[:]).then_inc(s, 16)
        if final_wait:
            nc.sync.wait_ge(s, 67)

    return nc


def _get_nc(**kwargs):
    key = tuple(sorted(kwargs.items()))
    if key not in _CACHE:
        _CACHE[key] = _build_nc(**kwargs)
    return _CACHE[key]


def _in_maps(trg_boundary: np.ndarray):
    tb = np.asarray(trg_boundary)
    assert tb.shape == (B, T), tb.shape
    tb8 = tb.astype(np.int8)  # values are 0/1
    P = BS * NCHUNK
    return [
        {"tb": tb8[c * BS : (c + 1) * BS].reshape(P, CH)}
        for c in range(N_CORES)
    ]


def run_device(trg_boundary, nc_kwargs=None, **run_kwargs):
    """Compile (cached) + run on cores 0-7; returns BassKernelResults."""
    from concourse.bass_utils import run_bass_kernel_spmd

    return run_bass_kernel_spmd(
        _get_nc(**(nc_kwargs or {})),
        _in_maps(trg_boundary),
        core_ids=list(range(N_CORES)),
        **run_kwargs,
    )


def kernel(src_sent, trg_sent, src_boundary, trg_boundary):
    res = run_device(trg_boundary)
    tb = np.asarray(trg_boundary)
    off = np.arange(NCHUNK, dtype=np.float64) * CH  # chunk base offsets
    total = np.float64(0.0)
    for c, r in enumerate(res.results):
        o = np.asarray(r["out"], dtype=np.float64)  # [128, 2]
        m = o[:, 0].reshape(BS, NCHUNK)
        cnt = o[:, 1].reshape(BS, NCHUNK)
        lastp1 = np.where(m > 0, m + off, 0.0).max(axis=1)
        count = cnt.sum(axis=1)
        first = tb[c * BS : (c + 1) * BS, 0].astype(np.float64)
        sum_scales = count - first - np.maximum(lastp1, 1.0) + T + 1
        total += sum_scales.sum()
    return np.asarray(total * NEG_LOG_K, dtype=np.float32)
